# revision 1
# baseline (speedup 1.0000x reference)
"""Causal single-head attention (4096x2048, d=128) on 8 TRN2 NeuronCores.

Strategy (flash-style sequence parallelism):
- Q rows are sharded mod-8 across cores: core c owns global rows c::8.
  With causal masking this gives every core an *identical* work profile,
  so one SPMD program needs no per-core control flow.
- K/V projections are sharded by contiguous 512-key blocks; the projected
  K^T and V (bf16, 1MB each) are AllGathered instead of replicating the
  64MB fp32 K/V input streams on every core.
- Scores are computed transposed ([keys, rows] = K^T tiles as stationary,
  Q^T as moving) so exp(P) tiles feed the P@V matmul directly as the
  stationary operand with no on-chip transposes of P.
- The softmax denominator is obtained by appending a ones-column to V:
  out_psum[:, 0] accumulates sum_s P[s, r] alongside P@V.
- Causal mask is multiplicative (0/1) after exp, applied only to the 8
  diagonal-band key tiles per query tile; the mask pattern is independent
  of the query-tile index and is passed per-core from the host.
- K^T is gathered in fp8e4 (halves the K AllGather; score error smoothed
  by the softmax), V stays bf16 (V precision passes straight to the
  output). All matmuls are bf16 with fp32 PSUM accumulation.
- Attention runs scores+exp for all groups first (needs only K), then all
  P@V accumulations (needs V), so the V AllGather overlaps the score
  phase. With reps>1 the emission is software-pipelined: iteration i's
  attention is emitted after iteration i+1's projections+collectives,
  giving each AllGather a full phase of latency slack in the in-order
  engine streams.
"""

import math
import sys

sys.path.insert(0, "/opt/trn_rl_repo")

import ml_dtypes
import numpy as np

import concourse.bass as bass
import concourse.tile as tile
from concourse import bacc, mybir
from concourse.bass import ts
from concourse.bass_utils import run_bass_kernel_spmd
from concourse.masks import make_identity

N_CORES = 8
SEQ = 4096
D_MODEL = 2048
D_HEAD = 128
R = SEQ // N_CORES          # 512 query rows per core
KB = SEQ // N_CORES         # 512 keys projected per core
N_QT = R // 128             # 4 query tiles of 128 rows per core
N_MT = D_MODEL // 128       # 16 contraction tiles for projections
N_KT = SEQ // 128           # 32 key tiles total
INV_SQRT = 1.0 / math.sqrt(D_MODEL)
VSTRIDE = 130               # V tile stride in Vaug (1 ones-col + 128 dims + pad)

BF16 = mybir.dt.bfloat16
F32 = mybir.dt.float32


def _build(reps=1, do_proj=True, do_coll=True, do_attn=True, do_load=True, single=False, cc_frac=1):
    n_dev = 1 if single else N_CORES
    nc = bacc.Bacc("TRN2", target_bir_lowering=False, debug=False,
                   num_devices=n_dev)

    iq_t = nc.dram_tensor("iq_t", [D_MODEL, R], BF16, kind="ExternalInput").ap()
    ik_t = nc.dram_tensor("ik_t", [D_MODEL, KB], BF16, kind="ExternalInput").ap()
    iv_t = nc.dram_tensor("iv_t", [D_MODEL, KB], BF16, kind="ExternalInput").ap()
    wq_t = nc.dram_tensor("wq_t", [D_MODEL, D_HEAD], BF16, kind="ExternalInput").ap()
    wk_t = nc.dram_tensor("wk_t", [D_MODEL, D_HEAD], BF16, kind="ExternalInput").ap()
    wv_t = nc.dram_tensor("wv_t", [D_MODEL, D_HEAD], BF16, kind="ExternalInput").ap()
    biases = nc.dram_tensor("biases", [D_HEAD, 3], F32, kind="ExternalInput").ap()
    mask = nc.dram_tensor("mask", [128, 8 * 128], BF16, kind="ExternalInput").ap()
    out = nc.dram_tensor("out", [R, D_HEAD], F32, kind="ExternalOutput").ap()

    with tile.TileContext(nc) as tc:
        with (
            tc.tile_pool(name="const", bufs=1) as const,
            tc.tile_pool(name="inbuf", bufs=6) as inbuf,
            tc.tile_pool(name="work", bufs=3) as work,
            tc.tile_pool(name="psum", bufs=2, space="PSUM") as psum,
            tc.tile_pool(name="dram", bufs=1, space="DRAM") as dram,
        ):
            # ---- constants ----
            ident = const.tile([128, 128], BF16)
            make_identity(nc, ident)
            # weights first (wk gates the first projection matmuls):
            # [128, 16*128], m-tile t at cols [128t, 128t+128)
            w_sbs = {}
            for name, wdram in (("wk", wk_t), ("wv", wv_t), ("wq", wq_t)):
                w_sb = const.tile([128, D_MODEL], BF16, name=f"{name}_sb")
                nc.scalar.dma_start(
                    w_sb.rearrange("p (t d) -> p t d", t=N_MT),
                    wdram.rearrange("(t p) d -> p t d", t=N_MT),
                )
                w_sbs[name] = w_sb
            b_sb = const.tile([D_HEAD, 3], F32)
            nc.scalar.dma_start(b_sb[:], biases[:])
            bq_sb, bk_sb, bv_sb = b_sb[:, 0:1], b_sb[:, 1:2], b_sb[:, 2:3]
            mask_sb = const.tile([128, 8 * 128], BF16)
            nc.scalar.dma_start(mask_sb[:], mask[:])
            # Vaug: 32 V tiles at stride VSTRIDE; col 0 of each is the ones
            # column for the softmax denominator (set once, V cols rewritten
            # each iteration).
            va_slots = []
            for vi in range(2):
                vaug_sb = const.tile([128, N_KT * VSTRIDE], BF16,
                                     name=f"vaug{vi}_sb")
                nc.vector.memset(vaug_sb[:], 1.0)
                va_slots.append(vaug_sb.rearrange("p (t x) -> p t x", t=N_KT))

            # ---- everything below repeats `reps` times (benchmarking aid;
            # reps=1 for production). Pool slot reuse serializes iterations.
            CHUNK = 4   # m-tiles per input DMA

            def project(w_sb, x_dram, ncols, bias_sb, out_sb, eng=None):
                eng = eng or nc.sync
                xt = []
                for ch in range(N_MT // CHUNK):
                    x_in = inbuf.tile([128, CHUNK * ncols], BF16,
                                      name="x_in", tag="xin")
                    eng.dma_start(
                        x_in.rearrange("p (t s) -> p t s", t=CHUNK),
                        x_dram[ts(ch, CHUNK * 128), :]
                        .rearrange("(t p) s -> p t s", t=CHUNK),
                    )
                    xt.append(x_in)
                pp = psum.tile([128, ncols], F32, name="proj_ps", tag="sc",
                               padded_shape=[128, 1024], bufs=2)
                for t in range(N_MT):
                    nc.tensor.matmul(
                        pp[:], w_sb[:, ts(t, 128)],
                        xt[t // CHUNK][:, ts(t % CHUNK, ncols)],
                        start=(t == 0), stop=(t == N_MT - 1))
                # PSUM -> SBUF bf16 with per-partition bias add
                nc.vector.tensor_scalar_add(out_sb[:], pp[:], bias_sb[:])

            def proj_phase(it):
                F8 = mybir.dt.float8e4
                kt_sb = work.tile([128, KB], F8, bufs=2, name="kt_sb")
                vt_sb = work.tile([128, KB], BF16, bufs=2, name="vt_sb")
                qt_sb = work.tile([128, R], BF16, bufs=2, name="qt_sb")
                if do_proj:
                    project(w_sbs["wk"], ik_t, KB, bk_sb, kt_sb)
                    project(w_sbs["wv"], iv_t, KB, bv_sb, vt_sb)
                    project(w_sbs["wq"], iq_t, R, bq_sb, qt_sb, eng=nc.scalar)

                # ---- AllGather K^T (fp8) ----
                cc_k_in = dram.tile([128, KB], F8, bufs=2, name="cc_k_in")
                cc_k_out = dram.tile([N_CORES, 128, KB], F8,
                                     addr_space="Shared", bufs=2,
                                     name="cc_k_out")
                if do_proj:
                    nc.sync.dma_start(cc_k_in[:], kt_sb[:])
                if do_coll and single:
                    nc.sync.dma_start(cc_k_out[0], cc_k_in[:])
                elif do_coll:
                    nc.gpsimd.collective_compute(
                        "AllGather", mybir.AluOpType.bypass,
                        replica_groups=[list(range(N_CORES))],
                        ins=[cc_k_in.opt()], outs=[cc_k_out.opt()],
                    )

                # ---- transpose V^T -> V block rows, AllGather V (bf16) ----
                vtr_sb = work.tile([128, KB], BF16, bufs=2, name="vtr_sb")
                for t in range(KB // 128 if do_proj else 0):
                    tp = psum.tile([128, 128], BF16, name="tp_ps", tag="tp",
                                   bufs=1)
                    nc.tensor.transpose(tp[:], vt_sb[:, ts(t, 128)], ident[:])
                    nc.vector.tensor_copy(vtr_sb[:, ts(t, 128)], tp[:])
                cc_v_in = dram.tile([KB, 128], BF16, bufs=2, name="cc_v_in")
                cc_v_out = dram.tile([N_CORES, KB, 128], BF16,
                                     addr_space="Shared", bufs=2,
                                     name="cc_v_out")
                if do_proj:
                    nc.sync.dma_start(
                        cc_v_in.rearrange("(t p) d -> p t d", t=KB // 128),
                        vtr_sb.rearrange("p (t d) -> p t d", t=KB // 128),
                    )
                if do_coll and single:
                    nc.sync.dma_start(cc_v_out[0], cc_v_in[:])
                elif do_coll:
                    nc.gpsimd.collective_compute(
                        "AllGather", mybir.AluOpType.bypass,
                        replica_groups=[list(range(N_CORES))],
                        ins=[cc_v_in.opt()], outs=[cc_v_out.opt()],
                    )
                return qt_sb, cc_k_out, cc_v_out

            def attn_phase(it, qt_sb, cc_k_out, cc_v_out):
                F8 = mybir.dt.float8e4
                if not do_load:
                    return
                ktf8_sb = const.tile([128, SEQ], F8, name="ktf8_sb", bufs=2)
                ktf_sb = const.tile([128, SEQ], BF16, name="ktf_sb", bufs=2)
                nc.sync.dma_start(
                    ktf8_sb.rearrange("p (r s) -> p r s", r=N_CORES),
                    cc_k_out.rearrange("r p s -> p r s"),
                )
                for h in range(2):
                    nc.vector.tensor_copy(ktf_sb[:, ts(h, SEQ // 2)],
                                          ktf8_sb[:, ts(h, SEQ // 2)])
                va = va_slots[it % 2]
                nc.scalar.dma_start(
                    va[:, :, 1:129],
                    cc_v_out.rearrange("r (t p) d -> p (r t) d", t=KB // 128),
                )

                # scores-first: all score groups + exp (needs only K^T);
                # then all P@V accumulations (needs V).
                if not do_attn:
                    return
                p_all = work.tile([128, 10 * 1024], BF16, bufs=2, name="p_all")
                slot = 0
                slots = {}
                for j in range(N_QT):
                    for g in range(j + 1):
                        spsum = psum.tile([128, 1024], F32, name="spsum",
                                          tag="sc")
                        for q in range(8):
                            kt = 8 * g + q
                            nc.tensor.matmul(
                                spsum[:, ts(q, 128)],
                                ktf_sb[:, ts(kt, 128)],
                                qt_sb[:, ts(j, 128)],
                                start=True, stop=True,
                            )
                        p_sb = p_all[:, ts(slot, 1024)]
                        nc.scalar.activation(p_sb, spsum[:],
                                             mybir.ActivationFunctionType.Exp,
                                             scale=INV_SQRT)
                        if g == j:
                            nc.vector.tensor_mul(p_sb, p_sb, mask_sb[:])
                        slots[(j, g)] = slot
                        slot += 1
                for j in range(N_QT):
                    opsum = psum.tile([128, 129], F32, name="opsum", tag="acc")
                    for g in range(j + 1):
                        p_sb = p_all[:, ts(slots[(j, g)], 1024)]
                        for q in range(8):
                            kt = 8 * g + q
                            nc.tensor.matmul(
                                opsum[:],
                                p_sb[:, ts(q, 128)].opt(),
                                va[:, kt, 0:129],
                                start=(g == 0 and q == 0),
                                stop=(g == j and q == 7),
                            )
                    recip = work.tile([128, 1], F32, name="recip", tag="recip")
                    nc.vector.reciprocal(recip[:], opsum[:, 0:1])
                    o_sb = work.tile([128, D_HEAD], F32, name="o_sb", tag="o")
                    nc.vector.tensor_scalar_mul(o_sb[:], opsum[:, 1:129],
                                                recip[:])
                    nc.scalar.dma_start(out[ts(j, 128), :], o_sb[:])

            # software pipeline: iteration i's attention is emitted after
            # iteration i+1's projections+collectives, so the in-order engine
            # streams give each AllGather a full phase of latency slack.
            pending = None
            for it in range(reps):
                handles = proj_phase(it)
                if pending is not None:
                    attn_phase(pending[0], *pending[1])
                pending = (it, handles)
            attn_phase(pending[0], *pending[1])

    nc.compile()
    return nc


_NC_CACHE = None


def _get_nc():
    global _NC_CACHE
    if _NC_CACHE is None:
        _NC_CACHE = _build()
    return _NC_CACHE


def make_in_maps(input_q, input_k, input_v, WQ_w, WQ_b, WK_w, WK_b, WV_w, WV_b):
    bf16 = ml_dtypes.bfloat16
    input_q = np.asarray(input_q, dtype=np.float32)
    input_k = np.asarray(input_k, dtype=np.float32)
    input_v = np.asarray(input_v, dtype=np.float32)
    wq_t = np.ascontiguousarray(np.asarray(WQ_w, np.float32).T).astype(bf16)
    wk_t = np.ascontiguousarray(np.asarray(WK_w, np.float32).T).astype(bf16)
    wv_t = np.ascontiguousarray(np.asarray(WV_w, np.float32).T).astype(bf16)
    biases_h = np.stack([
        np.asarray(WQ_b, np.float32),
        np.asarray(WK_b, np.float32),
        np.asarray(WV_b, np.float32),
    ], axis=1)  # [128, 3]

    s = np.arange(128)[:, None, None]     # s_local (key within tile)
    m = np.arange(8)[None, :, None]       # diag-band key tile index
    r = np.arange(128)[None, None, :]     # r_local (query within tile)
    in_maps = []
    for c in range(N_CORES):
        mask_c = np.ascontiguousarray(
            ((128 * m + s) <= (8 * r + c)).transpose(0, 1, 2)
        ).astype(bf16).reshape(128, 1024)
        in_maps.append({
            "iq_t": np.ascontiguousarray(input_q[c::8].T).astype(bf16),
            "ik_t": np.ascontiguousarray(input_k[KB * c:KB * (c + 1)].T).astype(bf16),
            "iv_t": np.ascontiguousarray(input_v[KB * c:KB * (c + 1)].T).astype(bf16),
            "wq_t": wq_t, "wk_t": wk_t, "wv_t": wv_t,
            "biases": biases_h,
            "mask": mask_c,
        })
    return in_maps


def assemble(results):
    full = np.empty((SEQ, D_HEAD), dtype=np.float32)
    for c in range(N_CORES):
        full[c::8] = results[c]["out"]
    return full


def kernel(**inputs):
    nc = _get_nc()
    in_maps = make_in_maps(**inputs)
    try:
        res = run_bass_kernel_spmd(nc, in_maps, core_ids=list(range(N_CORES)))
    except Exception:
        # The axon-tunneled devices occasionally report a transient
        # NRT_EXEC_UNIT_UNRECOVERABLE fault left over from a previous
        # session; a single retry has been observed to clear it.
        import time as _time
        _time.sleep(2.0)
        res = run_bass_kernel_spmd(nc, in_maps, core_ids=list(range(N_CORES)))
    return assemble(res.results)



# revision 6
# speedup vs baseline: 1.0959x; 1.0959x over previous
"""Causal single-head attention (4096x2048, d=128) on 8 TRN2 NeuronCores.

Strategy (flash-style sequence parallelism, v2):
- Q rows sharded mod-8 across cores (identical causal work profile per
  core); K/V projections sharded by contiguous 512-key blocks, projected
  K^T (fp8e4) and V (bf16->fp8?no: bf16) AllGathered.
- Xq/Xk inputs and Wq/Wk weights are quantized to fp8e4 on the host
  (weights pre-scaled by 64 to stay in normal fp8 range); the Q/K
  projections run in DoubleRow fp8 perf mode (2 fp8 MACs/cell/cycle,
  contraction 256 per matmul) - about 1.8x the bf16 projection rate and
  half the input DMA traffic.
- V projection is computed X-stationary: out[s,d] = sum_m X^T[m,s]^T W^T[m,d]
  so V lands directly in [keys, d] layout - no PE transposes - and the
  gather input is written contiguously.  V stays bf16 end-to-end (fp8 V
  costs ~2.6e-2 rel err, over the 2e-2 budget).
- The softmax-denominator ones-column is carried *inside the gathered V
  blocks* (each core writes [128, 4, 1+128] with col 0 = 1.0), so the
  gathered V streams straight into PV matmuls with FD=129 and no
  receiver-side fixups, and the gather reload runs 1032B-contiguous.
- Gathered K^T is consumed directly as the fp8 stationary operand of the
  score matmuls (mixed fp8 x bf16 matmul) - no on-chip upconvert.
- All inputs are host-swizzled into the exact SBUF layout ([128, t, n],
  m = 128 t + p) so every input DMA is fully contiguous per partition.
- Emission is software-pipelined 3 deep (loads(it+1) | proj(it) |
  attn(it-1)) and the PE stream interleaves score-group matmuls of
  attn(it-1) between projection chunks of proj(it), so the ScalarE exp
  chain (~11.5us/iter, the score-phase pacer) runs entirely under PE
  projection/PV work instead of stalling it.
"""

import math
import sys

sys.path.insert(0, "/opt/trn_rl_repo")

import ml_dtypes
import numpy as np

import concourse.bass as bass
import concourse.tile as tile
from concourse import bacc, mybir
from concourse.bass import ts
from concourse.bass_utils import run_bass_kernel_spmd

N_CORES = 8
SEQ = 4096
D_MODEL = 2048
D_HEAD = 128
R = SEQ // N_CORES          # 512 query rows per core
KB = SEQ // N_CORES         # 512 keys projected per core
N_QT = R // 128             # 4 query tiles of 128 rows per core
N_MT = D_MODEL // 128       # 16 contraction tiles for projections
N_KT = SEQ // 128           # 32 key tiles total
INV_SQRT = 1.0 / math.sqrt(D_MODEL)
WSCALE = 64.0               # fp8 weight pre-scale (host side)
VW = 129                    # gathered V block width: 1 ones-col + 128 dims

BF16 = mybir.dt.bfloat16
F32 = mybir.dt.float32
F8 = mybir.dt.float8e4
DR = mybir.MatmulPerfMode.DoubleRow


def _build(reps=1):
    nc = bacc.Bacc("TRN2", target_bir_lowering=False, debug=False,
                   num_devices=N_CORES)

    xq_d = nc.dram_tensor("xq8", [128, N_MT * R], F8, kind="ExternalInput").ap()
    xk_d = nc.dram_tensor("xk8", [128, N_MT * KB], F8, kind="ExternalInput").ap()
    xv_d = nc.dram_tensor("xv", [128, N_MT * KB], BF16, kind="ExternalInput").ap()
    wq_d = nc.dram_tensor("wq8", [128, N_MT * D_HEAD], F8, kind="ExternalInput").ap()
    wk_d = nc.dram_tensor("wk8", [128, N_MT * D_HEAD], F8, kind="ExternalInput").ap()
    wv_d = nc.dram_tensor("wv", [128, N_MT * D_HEAD], BF16, kind="ExternalInput").ap()
    qkb_d = nc.dram_tensor("qkbias", [D_HEAD, 2], F32, kind="ExternalInput").ap()
    vb_d = nc.dram_tensor("vbias", [128, 4 * D_HEAD], BF16, kind="ExternalInput").ap()
    mask_d = nc.dram_tensor("mask", [128, 8 * 128], BF16, kind="ExternalInput").ap()
    out_d = nc.dram_tensor("out", [R, D_HEAD], BF16, kind="ExternalOutput").ap()

    with tile.TileContext(nc) as tc:
        with (
            tc.tile_pool(name="const", bufs=1) as const,
            tc.tile_pool(name="xin", bufs=2) as xin,
            tc.tile_pool(name="work", bufs=2) as work,
            tc.tile_pool(name="psum", bufs=2, space="PSUM") as psum,
            tc.tile_pool(name="dram", bufs=2, space="DRAM") as dram,
        ):
            # ---- constants (loaded once, amortized across reps) ----
            wq8 = const.tile([128, N_MT, D_HEAD], F8, name="wq8")
            nc.sync.dma_start(wq8[:], wq_d.rearrange("p (t d) -> p t d", t=N_MT))
            wk8 = const.tile([128, N_MT, D_HEAD], F8, name="wk8")
            nc.sync.dma_start(wk8[:], wk_d.rearrange("p (t d) -> p t d", t=N_MT))
            wv_sb = const.tile([128, N_MT, D_HEAD], BF16, name="wv_sb")
            nc.sync.dma_start(wv_sb[:], wv_d.rearrange("p (t d) -> p t d", t=N_MT))
            qkb = const.tile([D_HEAD, 2], F32, name="qkb")
            nc.sync.dma_start(qkb[:], qkb_d[:])
            vbias = const.tile([128, 4 * D_HEAD], BF16, name="vbias")
            nc.sync.dma_start(vbias[:], vb_d[:])
            mask_sb = const.tile([128, 8 * 128], BF16, name="mask_sb")
            nc.sync.dma_start(mask_sb[:], mask_d[:])

            def emit_loads(it):
                xq = xin.tile([128, N_MT, R], F8, name="xq", tag="xq")
                nc.sync.dma_start(
                    xq[:], xq_d.rearrange("p (t r) -> p t r", t=N_MT))
                xk = xin.tile([128, N_MT, KB], F8, name="xk", tag="xk")
                nc.scalar.dma_start(
                    xk[:], xk_d.rearrange("p (t s) -> p t s", t=N_MT))
                xv = xin.tile([128, N_MT, KB], BF16, name="xv", tag="xv")
                xv_src = xv_d.rearrange("p (t s) -> p t s", t=N_MT)
                nc.sync.dma_start(xv[:, 0:8], xv_src[:, 0:8])
                nc.scalar.dma_start(xv[:, 8:16], xv_src[:, 8:16])
                return (xq, xk, xv)

            def make_proj_chunks(it, loads):
                """8 emission chunks for iteration `it`'s projections +
                collectives; returns (chunks, handles-for-attn)."""
                xq, xk, xv = loads
                qt = work.tile([128, R], BF16, name="qt", tag="qt")
                kt8 = work.tile([128, KB], F8, name="kt8", tag="kt8")
                vt = work.tile([128, 4, VW], BF16, name="vt", tag="vt")
                ktf8 = work.tile([128, N_CORES, KB], F8, name="ktf8", tag="ktf8")
                va = work.tile([128, N_KT, VW], BF16, name="va", tag="va")

                cc_k_in = dram.tile([128, KB], F8, name="cc_k_in")
                cc_k_out = dram.tile([N_CORES, 128, KB], F8,
                                     addr_space="Shared", name="cc_k_out")
                cc_v_in = dram.tile([128, 4 * VW], BF16, name="cc_v_in")
                cc_v_out = dram.tile([N_CORES, 128, 4 * VW], BF16,
                                     addr_space="Shared", name="cc_v_out")

                state = {}

                def q0():
                    pq = psum.tile([128, R], F32, name="pq", tag="proj")
                    state["pq"] = pq
                    for i in range(4):
                        nc.tensor.matmul(
                            pq[:], wq8[:, 2 * i:2 * i + 2, :],
                            xq[:, 2 * i:2 * i + 2, :],
                            start=(i == 0), stop=False, perf_mode=DR)

                def q1():
                    pq = state["pq"]
                    for i in range(4, 8):
                        nc.tensor.matmul(
                            pq[:], wq8[:, 2 * i:2 * i + 2, :],
                            xq[:, 2 * i:2 * i + 2, :],
                            start=False, stop=(i == 7), perf_mode=DR)
                    nc.vector.tensor_scalar(
                        qt[:], pq[:], 1.0 / WSCALE, qkb[:, 0:1],
                        op0=mybir.AluOpType.mult, op1=mybir.AluOpType.add)

                def k0():
                    pk = psum.tile([128, KB], F32, name="pk", tag="proj")
                    state["pk"] = pk
                    for i in range(4):
                        nc.tensor.matmul(
                            pk[:], wk8[:, 2 * i:2 * i + 2, :],
                            xk[:, 2 * i:2 * i + 2, :],
                            start=(i == 0), stop=False, perf_mode=DR)

                def k1():
                    pk = state["pk"]
                    for i in range(4, 8):
                        nc.tensor.matmul(
                            pk[:], wk8[:, 2 * i:2 * i + 2, :],
                            xk[:, 2 * i:2 * i + 2, :],
                            start=False, stop=(i == 7), perf_mode=DR)
                    nc.vector.tensor_scalar(
                        kt8[:], pk[:], 1.0 / WSCALE, qkb[:, 1:2],
                        op0=mybir.AluOpType.mult, op1=mybir.AluOpType.add)
                    nc.sync.dma_start(cc_k_in[:], kt8[:])
                    nc.gpsimd.collective_compute(
                        "AllGather", mybir.AluOpType.bypass,
                        replica_groups=[list(range(N_CORES))],
                        ins=[cc_k_in.opt()], outs=[cc_k_out.opt()],
                    )

                def make_v(b):
                    def v():
                        if b == 0:
                            state["pv"] = psum.tile([128, 512], F32,
                                                    name="pv", tag="proj")
                        pv = state["pv"]
                        for t in range(N_MT):
                            nc.tensor.matmul(
                                pv[:, ts(b, 128)], xv[:, t, :][:, ts(b, 128)],
                                wv_sb[:, t, :],
                                start=(t == 0), stop=(t == N_MT - 1))
                        if b == 3:
                            nc.vector.tensor_add(
                                vt[:, :, 1:VW],
                                pv.rearrange("p (t d) -> p t d", t=4),
                                vbias.rearrange("p (t d) -> p t d", t=4))
                            nc.vector.memset(vt[:, :, 0:1], 1.0)
                            nc.sync.dma_start(
                                cc_v_in[:],
                                vt.rearrange("p t d -> p (t d)"))
                            nc.gpsimd.collective_compute(
                                "AllGather", mybir.AluOpType.bypass,
                                replica_groups=[list(range(N_CORES))],
                                ins=[cc_v_in.opt()], outs=[cc_v_out.opt()],
                            )
                    return v

                # Reload chunks are issued from gpsimd (whose stream holds
                # only the collectives), so their wait-on-collective blocks
                # nothing; issuing them from sync/scalar would head-of-line
                # block next-tick loads/exps behind the CC wait.
                def reload_k():
                    nc.gpsimd.dma_start(
                        ktf8[:], cc_k_out.rearrange("r p s -> p r s"))

                def reload_v():
                    nc.gpsimd.dma_start(
                        va.rearrange("p (r t) x -> p r (t x)", r=N_CORES),
                        cc_v_out.rearrange("r p x -> p r x"))

                chunks = [q0, q1, k0, k1] + [make_v(b) for b in range(4)]
                return chunks, (reload_k, reload_v), (qt, ktf8, va)

            def make_attn_chunks(it, handles):
                """14 emission chunks for iteration `it`'s attention."""
                qt, ktf8, va = handles
                ktf = ktf8.rearrange("p r s -> p (r s)")
                p_all = work.tile([128, 10 * 1024], BF16, name="p_all",
                                  tag="p_all")
                slots = {}
                state = {"slot": 0}

                def make_sc(j, g):
                    def sc():
                        spsum = psum.tile([128, 1024], F32, name="spsum",
                                          tag="sc")
                        for q in range(8):
                            kt = 8 * g + q
                            nc.tensor.matmul(
                                spsum[:, ts(q, 128)],
                                ktf[:, ts(kt, 128)],
                                qt[:, ts(j, 128)],
                                start=True, stop=True)
                        slot = state["slot"]
                        state["slot"] += 1
                        slots[(j, g)] = slot
                        p_sb = p_all[:, ts(slot, 1024)]
                        nc.scalar.activation(p_sb, spsum[:],
                                             mybir.ActivationFunctionType.Exp,
                                             scale=INV_SQRT)
                        if g == j:
                            nc.vector.tensor_mul(p_sb, p_sb, mask_sb[:])
                    return sc

                def make_pv(j):
                    def pv():
                        opsum = psum.tile([128, VW], F32, name="opsum",
                                          tag="acc")
                        for g in range(j + 1):
                            p_sb = p_all[:, ts(slots[(j, g)], 1024)]
                            for q in range(8):
                                kt = 8 * g + q
                                nc.tensor.matmul(
                                    opsum[:],
                                    p_sb[:, ts(q, 128)].opt(),
                                    va[:, kt, :],
                                    start=(g == 0 and q == 0),
                                    stop=(g == j and q == 7))
                        recip = work.tile([128, 1], F32, name="recip",
                                          tag="recip")
                        nc.vector.reciprocal(recip[:], opsum[:, 0:1])
                        o_sb = work.tile([128, D_HEAD], BF16, name="o_sb",
                                         tag="o")
                        nc.vector.tensor_scalar_mul(o_sb[:], opsum[:, 1:VW],
                                                    recip[:])
                        nc.sync.dma_start(out_d[ts(j, 128), :], o_sb[:])
                    return pv

                sc = {(j, g): make_sc(j, g) for j in range(N_QT)
                      for g in range(j + 1)}
                pv = [make_pv(j) for j in range(N_QT)]
                return [sc[(0, 0)], sc[(1, 0)], sc[(1, 1)], sc[(2, 0)],
                        sc[(2, 1)], sc[(2, 2)], sc[(3, 0)], sc[(3, 1)],
                        pv[0], sc[(3, 2)], pv[1], sc[(3, 3)], pv[2], pv[3]]

            # ---- 3-deep software pipeline driver ----
            loads = emit_loads(0)
            attn_prev = None
            for it in range(reps):
                next_loads = emit_loads(it + 1) if it + 1 < reps else None
                pchunks, (rel_k, rel_v), handles = make_proj_chunks(it, loads)
                achunks = make_attn_chunks(it - 1, attn_prev) \
                    if attn_prev is not None else []
                if achunks:
                    # interleave: sc groups between proj chunks, PV at tail
                    seq = []
                    for i in range(8):
                        seq.append(achunks[i])
                        seq.append(pchunks[i])
                    seq += [rel_k] + achunks[8:] + [rel_v]
                else:
                    seq = pchunks + [rel_k, rel_v]
                for c in seq:
                    c()
                attn_prev = handles
                loads = next_loads
            for c in make_attn_chunks(reps - 1, attn_prev):
                c()

    nc.compile()
    return nc


_NC_CACHE = None


def _get_nc():
    global _NC_CACHE
    if _NC_CACHE is None:
        _NC_CACHE = _build()
    return _NC_CACHE


def _swizzle(a):
    """[2048, n] -> [128, 16*n] with row m = 128 t + p at [p, t*n : t*n+n]."""
    n = a.shape[1]
    return np.ascontiguousarray(
        a.reshape(N_MT, 128, n).transpose(1, 0, 2).reshape(128, N_MT * n))


def make_in_maps(input_q, input_k, input_v, WQ_w, WQ_b, WK_w, WK_b, WV_w, WV_b):
    bf16 = ml_dtypes.bfloat16
    f8 = ml_dtypes.float8_e4m3
    input_q = np.asarray(input_q, dtype=np.float32)
    input_k = np.asarray(input_k, dtype=np.float32)
    input_v = np.asarray(input_v, dtype=np.float32)
    wq8 = _swizzle(np.asarray(WQ_w, np.float32).T * WSCALE).astype(f8)
    wk8 = _swizzle(np.asarray(WK_w, np.float32).T * WSCALE).astype(f8)
    wv = _swizzle(np.asarray(WV_w, np.float32).T).astype(bf16)
    qkbias = np.stack([np.asarray(WQ_b, np.float32),
                       np.asarray(WK_b, np.float32)], axis=1)  # [128, 2]
    vbias = np.ascontiguousarray(np.broadcast_to(
        np.asarray(WV_b, np.float32)[None, None, :],
        (128, 4, D_HEAD)).reshape(128, 4 * D_HEAD)).astype(bf16)

    s = np.arange(128)[:, None, None]     # s_local (key within tile)
    m = np.arange(8)[None, :, None]       # diag-band key tile index
    r = np.arange(128)[None, None, :]     # r_local (query within tile)
    in_maps = []
    for c in range(N_CORES):
        mask_c = np.ascontiguousarray(
            ((128 * m + s) <= (8 * r + c))
        ).astype(bf16).reshape(128, 1024)
        in_maps.append({
            "xq8": _swizzle(input_q[c::8].T).astype(f8),
            "xk8": _swizzle(input_k[KB * c:KB * (c + 1)].T).astype(f8),
            "xv": _swizzle(input_v[KB * c:KB * (c + 1)].T).astype(bf16),
            "wq8": wq8, "wk8": wk8, "wv": wv,
            "qkbias": qkbias, "vbias": vbias,
            "mask": mask_c,
        })
    return in_maps


def assemble(results):
    full = np.empty((SEQ, D_HEAD), dtype=np.float32)
    for c in range(N_CORES):
        full[c::8] = results[c]["out"].astype(np.float32)
    return full


def kernel(**inputs):
    nc = _get_nc()
    in_maps = make_in_maps(**inputs)
    try:
        res = run_bass_kernel_spmd(nc, in_maps, core_ids=list(range(N_CORES)))
    except Exception:
        # The axon-tunneled devices occasionally report a transient
        # NRT_EXEC_UNIT_UNRECOVERABLE fault left over from a previous
        # session; a single retry has been observed to clear it.
        import time as _time
        _time.sleep(2.0)
        res = run_bass_kernel_spmd(nc, in_maps, core_ids=list(range(N_CORES)))
    return assemble(res.results)
